# revision 1
# baseline (speedup 1.0000x reference)
"""Cross-attention Trainium2 kernel (8 NeuronCores).

Sharding: batch (2) x head-groups (4 groups of 4 heads) = 8 shards.
Each core computes q/k/v projections for its 4 heads (256 cols of
Wq/Wk/Wv), attention for those heads, and a partial out-projection
through its 256 rows of Wo.  The host sums the 4 partial outputs per
batch (the reduction of the head-parallel out_proj) and adds the
bv @ Wo + bo correction, which commutes exactly through the softmax
average.

Layout strategy on-core:
  - x/ctx are PE-transposed (xT: [d, s]) so projections contract d on
    partitions; projections emit qT/kT ([head_dim, s]) and v (natural).
  - scores are computed transposed (ST = k @ qT -> [sk, sq]) so the
    exp'd tiles feed the attention matmul directly as the stationary
    operand; a ones-column in v gives the softmax denominator for free.
  - matmuls run as float32r (full-rate fp32, operands typed f32r
    end-to-end to satisfy walrus rounding rules); softmax P, v, attnT
    and Wo are fp16 (the fp32r ISA check forbids PSUM dst partition
    base 64, which the odd heads' attn transpose needs).
  - emission is software-pipelined: projections interleave with the
    first two windows' scores, each window's attn matmuls follow the
    next window's scores+exp, out_proj weaves into late windows.
"""

import numpy as np

import concourse.bass as bass
import concourse.mybir as mybir
import concourse.tile as tile
from concourse import bacc

B, SQ, SK, D, H, HS = 2, 2048, 2048, 1024, 16, 64
SCALE = HS ** -0.5
NCORES = 8
HG = 4            # heads per core
DG = HG * HS      # 256 projection cols per core

F32 = mybir.dt.float32
F32R = mybir.dt.float32r
F16 = mybir.dt.float16


def build_program(fast_mm: bool = True, pipeline: bool = True, loop_iters: int = 0):
    """Build the per-core SPMD Bass program."""
    MMDT = F32R if fast_mm else F32

    nc = bacc.Bacc(None, target_bir_lowering=False, debug=False,
                   num_devices=NCORES)
    x_d = nc.dram_tensor("x", [SQ, D], F16, kind="ExternalInput")
    c_d = nc.dram_tensor("ctx", [SK, D], F16, kind="ExternalInput")
    wq_d = nc.dram_tensor("wq", [D, DG], F16, kind="ExternalInput")
    wk_d = nc.dram_tensor("wk", [D, DG], F16, kind="ExternalInput")
    wv_d = nc.dram_tensor("wv", [D, DG], F16, kind="ExternalInput")
    wo_d = nc.dram_tensor("wo", [DG, D], F16, kind="ExternalInput")
    bq_d = nc.dram_tensor("bq", [DG], F32, kind="ExternalInput")
    bk_d = nc.dram_tensor("bk", [DG], F32, kind="ExternalInput")
    i_d = nc.dram_tensor("ident", [128, 128], MMDT, kind="ExternalInput")
    i16_d = nc.dram_tensor("ident16", [128, 128], F16, kind="ExternalInput")
    out_d = nc.dram_tensor("out", [SQ, D], F32, kind="ExternalOutput")

    with tile.TileContext(nc) as tc:
        with (
            tc.tile_pool(name="const", bufs=1) as cp,
            tc.tile_pool(name="persist", bufs=1) as psb,
            tc.tile_pool(name="xw", bufs=8) as xwp,
            tc.tile_pool(name="xtw", bufs=2) as xtwp,
            tc.tile_pool(name="expp", bufs=28) as ep,
            tc.tile_pool(name="fin", bufs=4) as fpool,
            tc.tile_pool(name="outp", bufs=3) as opool,
            tc.tile_pool(name="pp", bufs=2, space="PSUM") as pp,
            tc.tile_pool(name="stp", bufs=2, space="PSUM") as stp,
            tc.tile_pool(name="atp", bufs=2, space="PSUM") as atp,
        ):
            import contextlib
            loop_ctx = tc.For_i(0, loop_iters, 1) if loop_iters else contextlib.nullcontext()
            loop_ctx.__enter__()
            ident = cp.tile([128, 128], MMDT)
            nc.sync.dma_start(out=ident, in_=i_d[:])
            ident16 = cp.tile([128, 128], F16, tag="ident16")
            nc.sync.dma_start(out=ident16, in_=i16_d[:])

            wq_sb = cp.tile([128, 8, DG], F16, tag="wq")
            wk_sb = cp.tile([128, 8, DG], F16, tag="wk")
            wv_sb = cp.tile([128, 8, DG], F16, tag="wv")
            wo_sb = cp.tile([128, 2, D], F16, tag="wo")
            bq_sb = cp.tile([128, 2], F32, tag="bq")
            bk_sb = cp.tile([128, 2], F32, tag="bk")
            def load_weights_qx():
                nc.sync.dma_start(out=wq_sb, in_=wq_d[:].rearrange("(c p) n -> p c n", p=128))
                nc.sync.dma_start(out=bq_sb, in_=bq_d[:].rearrange("(c p) -> p c", p=128))

            def load_weights_kv():
                nc.sync.dma_start(out=wk_sb, in_=wk_d[:].rearrange("(c p) n -> p c n", p=128))
                nc.sync.dma_start(out=wv_sb, in_=wv_d[:].rearrange("(c p) n -> p c n", p=128))
                nc.sync.dma_start(out=bk_sb, in_=bk_d[:].rearrange("(c p) -> p c", p=128))

            def load_weights_o():
                nc.sync.dma_start(out=wo_sb, in_=wo_d[:].rearrange("(c p) n -> p c n", p=128))

            # persistent activations, split per producing window so the
            # scheduler's dependencies stay fine-grained
            qTs = [psb.tile([128, 2, 512], F16, tag=f"qT{w}", name=f"qT{w}") for w in range(4)]
            kTs = [psb.tile([128, 2, 512], F16, tag=f"kT{w}", name=f"kT{w}") for w in range(4)]
            vAs = [psb.tile([128, 4, HG, 68], F16, tag=f"vA{w}", name=f"vA{w}") for w in range(4)]
            aTs = [psb.tile([128, 2, 128], F16, tag=f"aT{s}", name=f"aT{s}") for s in range(16)]

            for w in range(4):
                nc.vector.memset(vAs[w][:], 1.0)

            def proj_window(src_d, dst_T, bias_sb, w_sb, with_v, w, after_dma=None):
                xts = []
                for i in range(4):
                    xt = xwp.tile([128, D], F16, tag="xw")
                    r0 = (w * 4 + i) * 128
                    nc.sync.dma_start(out=xt, in_=src_d[r0:r0 + 128, :])
                    xts.append(xt)
                if after_dma is not None:
                    after_dma()
                xtw = xtwp.tile([128, 8, 512], F16, tag="xtw")
                for dc in range(8):
                    pt = pp.tile([128, 512], F16, tag="pp")
                    for i in range(4):
                        nc.tensor.transpose(
                            (pt[:, i * 128:(i + 1) * 128]),
                            (xts[i][:, dc * 128:(dc + 1) * 128]),
                            (ident16),
                        )
                    nc.vector.tensor_copy(xtw[:, dc, :], pt)
                for c in range(2):
                    pq = pp.tile([128, 512], F32, tag="pp")
                    for dc in range(8):
                        nc.tensor.matmul(
                            pq,
                            (w_sb[:, dc, c * 128:(c + 1) * 128]),
                            (xtw[:, dc, :]),
                            start=(dc == 0), stop=(dc == 7),
                        )
                    nc.vector.tensor_scalar_add(
                        dst_T[w][:, c, :], pq, bias_sb[:, c:c + 1])
                if with_v:
                    for s in range(4):
                        # attention psum pool is idle during projections
                        pv = atp.tile([128, 512], F32, tag="at")
                        for dc in range(8):
                            nc.tensor.matmul(
                                pv[:, :DG],
                                (xtw[:, dc, s * 128:(s + 1) * 128]),
                                (wv_sb[:, dc, :]),
                                start=(dc == 0), stop=(dc == 7),
                            )
                        nc.vector.tensor_copy(
                            vAs[w][:, s, :, 0:64],
                            pv[:, :DG].rearrange("p (h e) -> p h e", e=64),
                        )

            def proj_x(w, after_dma=None):
                proj_window(x_d, qTs, bq_sb, wq_sb, False, w, after_dma)

            def proj_ctx(w, after_dma=None):
                proj_window(c_d, kTs, bk_sb, wk_sb, True, w, after_dma)

            # attention per head / sq-window of 1024, software-pipelined:
            # window w's attn-matmuls are emitted after window w+1's
            # scores+exp so ACT (exp) is never starved.
            def emit_scores_exp(h, sqw, skcs):
                p0 = 64 * (h % 2)
                t = h // 2
                exs = []
                for skc in skcs:
                    st = stp.tile([128, 1024], F32, tag="st")
                    for half in range(2):
                        qw = sqw * 2 + half
                        nc.tensor.matmul(
                            st[:, half * 512:(half + 1) * 512],
                            (kTs[skc // 4][p0:p0 + 64, t,
                                             (skc % 4) * 128:(skc % 4 + 1) * 128]),
                            (qTs[qw][p0:p0 + 64, t, :]),
                            start=True, stop=True,
                        )
                    ex = ep.tile([128, 1024], F16, tag="ex")
                    nc.scalar.activation(
                        ex, st, mybir.ActivationFunctionType.Exp,
                        scale=SCALE)
                    exs.append(ex)
                return exs

            def emit_attnv_fin(h, sqw, exs, per_j=None):
                p0 = 64 * (h % 2)
                t = h // 2
                # attn accumulation: one psum bank per sq-chunk j
                for j in range(8):
                    at = atp.tile([128, 512], F32, tag="at")
                    for skc in range(16):
                        nc.tensor.matmul(
                            at[:, 0:68],
                            exs[skc][:, j * 128:(j + 1) * 128],
                            vAs[skc // 4][:, skc % 4, h, :],
                            start=(skc == 0), stop=(skc == 15),
                        )
                    # normalize + transpose into aT
                    rc = fpool.tile([128, 1], F32, tag="rc")
                    nc.vector.reciprocal(rc, at[:, 64:65])
                    ad = fpool.tile([128, 64], F16, tag="ad")
                    nc.vector.tensor_scalar_mul(ad, at[:, 0:64], rc)
                    pt2 = pp.tile([128, 128], F16, tag="pp")
                    nc.tensor.transpose(pt2[p0:p0 + 64, :], ad, ident16)
                    nc.vector.tensor_copy(
                        aTs[sqw * 8 + j][p0:p0 + 64, t, :],
                        pt2[p0:p0 + 64, :])
                    if per_j is not None:
                        per_j(j)

            # out projection for a range of sq chunks (partial out: this
            # core's 256 attn cols)
            def emit_out_proj(sqcs):
                for sqc in sqcs:
                    ot = opool.tile([128, D], F32, tag="ot")
                    for n2 in range(2):
                        po = pp.tile([128, 512], F32, tag="pp")
                        for kc in range(2):
                            nc.tensor.matmul(
                                po,
                                (aTs[sqc][:, kc, :]),
                                (wo_sb[:, kc, n2 * 512:(n2 + 1) * 512]),
                                start=(kc == 0), stop=(kc == 1),
                            )
                        nc.vector.tensor_copy(ot[:, n2 * 512:(n2 + 1) * 512], po)
                    nc.sync.dma_start(
                        out=out_d[sqc * 128:(sqc + 1) * 128, :], in_=ot)

            if pipeline:
                # interleave projections with the first TWO attention
                # windows' scores so ACT (exp) starts as early as possible
                # (window 1 skc 8-15 depend on late ctx windows; window 2's
                # early skc only need ctx windows 0-1 and fill those gaps)
                proj_x(0, after_dma=load_weights_qx)
                proj_x(1)
                proj_ctx(0, after_dma=load_weights_kv)
                e1 = emit_scores_exp(0, 0, range(0, 4))
                proj_ctx(1)
                e1 += emit_scores_exp(0, 0, range(4, 8))
                e2 = emit_scores_exp(1, 0, range(0, 4))
                proj_x(2)
                proj_ctx(2)
                e1 += emit_scores_exp(0, 0, range(8, 12))
                e2 += emit_scores_exp(1, 0, range(4, 8))
                proj_x(3)
                proj_ctx(3)
                load_weights_o()
                e1 += emit_scores_exp(0, 0, range(12, 16))
                e2 += emit_scores_exp(1, 0, range(8, 12))
                emit_attnv_fin(0, 0, e1)
                e2 += emit_scores_exp(1, 0, range(12, 16))
                pending = (1, 0, e2)
                # out_proj chunks woven into the later (ACT-bound) windows
                op_after = {2: range(0, 2), 3: range(2, 4), 4: range(4, 6),
                            5: range(6, 8)}
                windows = [(h, sqw) for sqw in range(2) for h in range(HG)]
                for i, (h, sqw) in enumerate(windows[2:]):
                    exs = emit_scores_exp(h, sqw, range(0, 4))
                    emit_attnv_fin(*pending)
                    if i in op_after:
                        emit_out_proj(op_after[i])
                    pending = (h, sqw, exs)
                    exs += emit_scores_exp(h, sqw, range(4, 16))
                # final window: out_proj chunk 8+j right after its fin(j)
                emit_attnv_fin(*pending,
                               per_j=lambda j: emit_out_proj([8 + j]))
            else:
                load_weights_qx()
                load_weights_kv()
                load_weights_o()
                for w in range(4):
                    proj_x(w)
                for w in range(4):
                    proj_ctx(w)
                for h in range(HG):
                    for sqw in range(2):
                        exs = emit_scores_exp(h, sqw, range(16))
                        emit_attnv_fin(h, sqw, exs)
                emit_out_proj(range(16))
            loop_ctx.__exit__(None, None, None)

    nc.compile()
    return nc


_NC = None


def _program():
    global _NC
    if _NC is None:
        _NC = build_program()
    return _NC


def _f32(a):
    return np.ascontiguousarray(np.asarray(a, dtype=np.float32))


def kernel(inputs, context, Wq, bq, Wk, bk, Wv, bv, Wo, bo):
    from concourse.bass_utils import run_bass_kernel_spmd

    inputs = _f32(inputs)
    context = _f32(context)
    Wq, bq, Wk, bk = _f32(Wq), _f32(bq), _f32(Wk), _f32(bk)
    Wv, bv, Wo, bo = _f32(Wv), _f32(bv), _f32(Wo), _f32(bo)

    nc = _program()
    in_maps = []
    for core in range(NCORES):
        b, g = core // HG, core % HG
        sl = slice(DG * g, DG * (g + 1))
        in_maps.append({
            "x": np.ascontiguousarray(inputs[b].astype(np.float16)),
            "ctx": np.ascontiguousarray(context[b].astype(np.float16)),
            "wq": np.ascontiguousarray(Wq[:, sl].astype(np.float16)),
            "wk": np.ascontiguousarray(Wk[:, sl].astype(np.float16)),
            "wv": np.ascontiguousarray(Wv[:, sl].astype(np.float16)),
            "wo": np.ascontiguousarray(Wo[sl, :].astype(np.float16)),
            "bq": _f32(bq[sl]),
            "bk": _f32(bk[sl]),
            "ident": np.eye(128, dtype=np.float32),
            "ident16": np.eye(128, dtype=np.float16),
        })
    res = run_bass_kernel_spmd(nc, in_maps, list(range(NCORES)))
    outs = [res.results[i]["out"] for i in range(NCORES)]
    corr = (bv.astype(np.float64) @ Wo.astype(np.float64)
            + bo.astype(np.float64)).astype(np.float32)
    full = np.stack([
        outs[0] + outs[1] + outs[2] + outs[3],
        outs[4] + outs[5] + outs[6] + outs[7],
    ]) + corr
    return full.astype(np.float32)



# revision 3
# speedup vs baseline: 1.2255x; 1.2255x over previous
"""Cross-attention Trainium2 kernel (8 NeuronCores).

Sharding: batch (2) x head-groups (4 groups of 4 heads) = 8 shards.
Each core computes q/k/v projections for its 4 heads (256 cols of
Wq/Wk/Wv), attention for those heads, and a partial out-projection
through its 256 rows of Wo.  The host sums the 4 partial outputs per
batch (the reduction of the head-parallel out_proj) and adds the
bv @ Wo + bo correction, which commutes exactly through the softmax
average.

Layout strategy on-core (v2):
  - x/ctx arrive HOST-TRANSPOSED (xT: [d, s]) so no PE transposes are
    needed for the projections; projections emit qT/kT ([head_dim, s],
    head pairs stacked even/odd on partitions 0-63/64-127) and v
    natural ([s, head, hd] with a ones column for the softmax
    denominator).
  - scores are computed transposed (ST = k @ qT -> [sk, sq]); the
    even/odd head matmuls (K=64) are emitted adjacently so their
    auto-derived tile_positions (0,0)/(64,0) run concurrently on the
    two PE row-halves.
  - exp is written as fp8e4 so the attention matmuls' stationary
    loads hit 4-elem/cycle fast-weight-load; attention accumulates
    [sq,68] f32 in PSUM (col 64 = denominator), is normalized on DVE,
    and the head PAIR is transposed in one 128x128 PE transpose.
  - emission is software-pipelined: scores+exp of head-pair i+1
    interleave with the attention matmuls of pair i so ACT (exp, the
    bottleneck engine at ~134us busy) is never starved; out_proj
    chunks are woven in after their aT tiles complete.
"""

import numpy as np

import concourse.bass as bass
import concourse.mybir as mybir
import concourse.tile as tile
from concourse import bacc

B, SQ, SK, D, H, HS = 2, 2048, 2048, 1024, 16, 64
SCALE = HS ** -0.5
NCORES = 8
HG = 4            # heads per core
DG = HG * HS      # 256 projection cols per core

F32 = mybir.dt.float32
F16 = mybir.dt.float16
FP8 = mybir.dt.float8e4


def build_program(loop_iters: int = 0, fp8_exp: bool = True):
    """Build the per-core SPMD Bass program."""
    EXPDT = FP8 if fp8_exp else F16

    nc = bacc.Bacc(None, target_bir_lowering=False, debug=False,
                   num_devices=NCORES)
    x_d = nc.dram_tensor("xT", [D, SQ], F16, kind="ExternalInput")
    c_d = nc.dram_tensor("cT", [D, SK], F16, kind="ExternalInput")
    wq_d = nc.dram_tensor("wq", [D, DG], F16, kind="ExternalInput")
    wk_d = nc.dram_tensor("wk", [D, DG], F16, kind="ExternalInput")
    wv_d = nc.dram_tensor("wv", [D, DG], F16, kind="ExternalInput")
    wo_d = nc.dram_tensor("wo", [DG, D], F16, kind="ExternalInput")
    bq_d = nc.dram_tensor("bq", [DG], F32, kind="ExternalInput")
    bk_d = nc.dram_tensor("bk", [DG], F32, kind="ExternalInput")
    i16_d = nc.dram_tensor("ident16", [128, 128], F16, kind="ExternalInput")
    out_d = nc.dram_tensor("out", [SQ, D], F32, kind="ExternalOutput")

    with tile.TileContext(nc) as tc:
        with (
            tc.tile_pool(name="const", bufs=1) as cp,
            tc.tile_pool(name="persist", bufs=1) as psb,
            tc.tile_pool(name="xw", bufs=4) as xwp,
            tc.tile_pool(name="expp", bufs=68) as ep,
            tc.tile_pool(name="fin", bufs=4) as fpool,
            tc.tile_pool(name="outp", bufs=3) as opool,
            tc.tile_pool(name="pp", bufs=2, space="PSUM") as pp,
            tc.tile_pool(name="stp", bufs=2, space="PSUM") as stp,
            tc.tile_pool(name="atp", bufs=2, space="PSUM") as atp,
        ):
            import contextlib
            loop_ctx = tc.For_i(0, loop_iters, 1) if loop_iters else contextlib.nullcontext()
            loop_ctx.__enter__()
            ident16 = cp.tile([128, 128], F16, tag="ident16")
            nc.sync.dma_start(out=ident16, in_=i16_d[:])

            wq_sb = cp.tile([128, 8, DG], F16, tag="wq")
            wk_sb = cp.tile([128, 8, DG], F16, tag="wk")
            wv_sb = cp.tile([128, 8, DG], F16, tag="wv")
            wo_sb = cp.tile([128, 2, D], F16, tag="wo")
            bq_sb = cp.tile([128, 2], F32, tag="bq")
            bk_sb = cp.tile([128, 2], F32, tag="bk")

            def load_weights_qx():
                nc.sync.dma_start(out=wq_sb, in_=wq_d[:].rearrange("(c p) n -> p c n", p=128))
                nc.sync.dma_start(out=bq_sb, in_=bq_d[:].rearrange("(c p) -> p c", p=128))

            def load_weights_kv():
                nc.sync.dma_start(out=wk_sb, in_=wk_d[:].rearrange("(c p) n -> p c n", p=128))
                nc.sync.dma_start(out=wv_sb, in_=wv_d[:].rearrange("(c p) n -> p c n", p=128))
                nc.sync.dma_start(out=bk_sb, in_=bk_d[:].rearrange("(c p) -> p c", p=128))

            def load_weights_o():
                nc.sync.dma_start(out=wo_sb, in_=wo_d[:].rearrange("(c p) n -> p c n", p=128))

            # persistent activations: qT/kT hold head pairs stacked on
            # partitions (even head p0-63, odd head p64-127), pair index
            # on the middle axis, full sq/sk on the free axis
            qT = psb.tile([128, 2, SQ], F16, tag="qT", name="qT")
            kT = psb.tile([128, 2, SK], F16, tag="kT", name="kT")
            # v natural: [sk-chunk part, skc, head, 64+ones]
            vA = psb.tile([128, 16, HG, 68], F16, tag="vA", name="vA")
            aTs = [psb.tile([128, 2, 128], F16, tag=f"aT{s}", name=f"aT{s}")
                   for s in range(16)]

            nc.vector.memset(vA[:], 1.0)

            def proj_x(w, after_dma=None):
                xw = xwp.tile([128, 8, 512], F16, tag="xw")
                nc.sync.dma_start(
                    out=xw,
                    in_=x_d[:, w * 512:(w + 1) * 512]
                        .rearrange("(c p) s -> p c s", p=128))
                if after_dma is not None:
                    after_dma()
                for c in range(2):
                    pq = pp.tile([128, 512], F32, tag="pp")
                    for dc in range(8):
                        nc.tensor.matmul(
                            pq,
                            (wq_sb[:, dc, c * 128:(c + 1) * 128]),
                            (xw[:, dc, :]),
                            start=(dc == 0), stop=(dc == 7),
                        )
                    nc.vector.tensor_scalar_add(
                        qT[:, c, w * 512:(w + 1) * 512], pq, bq_sb[:, c:c + 1])

            def proj_ctx(w, after_dma=None):
                cw = xwp.tile([128, 8, 512], F16, tag="xw")
                nc.sync.dma_start(
                    out=cw,
                    in_=c_d[:, w * 512:(w + 1) * 512]
                        .rearrange("(c p) s -> p c s", p=128))
                if after_dma is not None:
                    after_dma()
                for c in range(2):
                    pk = pp.tile([128, 512], F32, tag="pp")
                    for dc in range(8):
                        nc.tensor.matmul(
                            pk,
                            (wk_sb[:, dc, c * 128:(c + 1) * 128]),
                            (cw[:, dc, :]),
                            start=(dc == 0), stop=(dc == 7),
                        )
                    nc.vector.tensor_scalar_add(
                        kT[:, c, w * 512:(w + 1) * 512], pk, bk_sb[:, c:c + 1])
                for s4 in range(4):
                    # attention psum pool is idle during projections
                    pv = atp.tile([128, 512], F32, tag="at")
                    for dc in range(8):
                        nc.tensor.matmul(
                            pv[:, :DG],
                            (cw[:, dc, s4 * 128:(s4 + 1) * 128]),
                            (wv_sb[:, dc, :]),
                            start=(dc == 0), stop=(dc == 7),
                        )
                    nc.vector.tensor_copy(
                        vA[:, w * 4 + s4, :, 0:64],
                        pv[:, :DG].rearrange("p (h e) -> p h e", e=64),
                    )

            # scores + exp for head pair t, sq window sqw, one sk chunk.
            # even/odd half-array matmuls are emitted adjacently so their
            # tile_positions (0,0)/(64,0) run concurrently on HW.
            def emit_se(t, sqw, skc):
                sts = [stp.tile([128, 1024], F32, tag="st", name=f"st{p}")
                       for p in range(2)]
                for half in range(2):
                    for par in range(2):
                        p0 = 64 * par
                        nc.tensor.matmul(
                            sts[par][:, half * 512:(half + 1) * 512],
                            (kT[p0:p0 + 64, t, skc * 128:(skc + 1) * 128]),
                            (qT[p0:p0 + 64, t,
                                sqw * 1024 + half * 512:
                                sqw * 1024 + (half + 1) * 512]),
                            start=True, stop=True,
                        )
                exs = []
                for par in range(2):
                    ex = ep.tile([128, 1024], EXPDT, tag="ex")
                    nc.scalar.activation(
                        ex, sts[par], mybir.ActivationFunctionType.Exp,
                        scale=SCALE)
                    exs.append(ex)
                return exs

            # attention j-chunk (128 sq rows) for head pair (t, sqw):
            # accumulate over all 16 sk chunks for both heads, normalize,
            # transpose the pair in one shot into aT
            def attn_j(t, sqw, exE, exO, j):
                ad2 = fpool.tile([128, 128], F16, tag="ad2")
                for par, exs in ((0, exE), (1, exO)):
                    at = atp.tile([128, 512], F32, tag="at")
                    for skc in range(16):
                        nc.tensor.matmul(
                            at[:, 0:68],
                            exs[skc][:, j * 128:(j + 1) * 128],
                            vA[:, skc, 2 * t + par, :],
                            start=(skc == 0), stop=(skc == 15),
                        )
                    rc = fpool.tile([128, 1], F32, tag="rc")
                    nc.vector.reciprocal(rc, at[:, 64:65])
                    nc.vector.tensor_scalar_mul(
                        ad2[:, par * 64:(par + 1) * 64], at[:, 0:64], rc)
                pt2 = pp.tile([128, 128], F16, tag="pp")
                nc.tensor.transpose(pt2, ad2, ident16)
                nc.vector.tensor_copy(aTs[sqw * 8 + j][:, t, :], pt2)

            # partial out-projection for one 128-row sq chunk
            def emit_out_proj(sqc):
                ot = opool.tile([128, D], F32, tag="ot")
                for n2 in range(2):
                    po = pp.tile([128, 512], F32, tag="pp")
                    for kc in range(2):
                        nc.tensor.matmul(
                            po,
                            (aTs[sqc][:, kc, :]),
                            (wo_sb[:, kc, n2 * 512:(n2 + 1) * 512]),
                            start=(kc == 0), stop=(kc == 1),
                        )
                    nc.vector.tensor_copy(ot[:, n2 * 512:(n2 + 1) * 512], po)
                nc.sync.dma_start(
                    out=out_d[sqc * 128:(sqc + 1) * 128, :], in_=ot)

            # ---- prologue: projections interleaved with the first pair's
            # scores so ACT starts as early as possible
            proj_x(0, after_dma=load_weights_qx)
            proj_x(1)
            proj_ctx(0, after_dma=load_weights_kv)
            e = {}
            P = [(0, 0), (1, 0), (0, 1), (1, 1)]
            se = [emit_se(0, 0, skc) for skc in range(4)]
            proj_ctx(1)
            se += [emit_se(0, 0, skc) for skc in range(4, 8)]
            proj_x(2)
            proj_ctx(2)
            se += [emit_se(0, 0, skc) for skc in range(8, 12)]
            proj_x(3)
            proj_ctx(3)
            load_weights_o()
            se += [emit_se(0, 0, skc) for skc in range(12, 16)]
            e[P[0]] = se

            # ---- main loop: attn(pair i) woven with scores+exp(pair i+1)
            for i in range(3):
                cur, nxt = P[i], P[i + 1]
                se = []
                for j in range(8):
                    se += [emit_se(*nxt, skc) for skc in (2 * j, 2 * j + 1)]
                    exE = [a for a, _ in e[cur]]
                    exO = [b for _, b in e[cur]]
                    attn_j(*cur, exE, exO, j)
                    if i == 1:
                        emit_out_proj(j)
                e[nxt] = se
            exE = [a for a, _ in e[P[3]]]
            exO = [b for _, b in e[P[3]]]
            for j in range(8):
                attn_j(*P[3], exE, exO, j)
                emit_out_proj(8 + j)
            loop_ctx.__exit__(None, None, None)

    nc.compile()
    return nc


_NC = None


def _program():
    global _NC
    if _NC is None:
        _NC = build_program()
    return _NC


def _f32(a):
    return np.ascontiguousarray(np.asarray(a, dtype=np.float32))


def make_in_maps(inputs, context, Wq, bq, Wk, bk, Wv, bv, Wo, bo):
    inputs = np.asarray(inputs)
    context = np.asarray(context)
    Wq, bq, Wk, bk = (np.asarray(a) for a in (Wq, bq, Wk, bk))
    Wv, Wo = np.asarray(Wv), np.asarray(Wo)
    in_maps = []
    for core in range(NCORES):
        b, g = core // HG, core % HG
        sl = slice(DG * g, DG * (g + 1))
        in_maps.append({
            "xT": np.ascontiguousarray(inputs[b].T.astype(np.float16)),
            "cT": np.ascontiguousarray(context[b].T.astype(np.float16)),
            "wq": np.ascontiguousarray(Wq[:, sl].astype(np.float16)),
            "wk": np.ascontiguousarray(Wk[:, sl].astype(np.float16)),
            "wv": np.ascontiguousarray(Wv[:, sl].astype(np.float16)),
            "wo": np.ascontiguousarray(Wo[sl, :].astype(np.float16)),
            "bq": _f32(bq[sl]),
            "bk": _f32(bk[sl]),
            "ident16": np.eye(128, dtype=np.float16),
        })
    return in_maps


def kernel(inputs, context, Wq, bq, Wk, bk, Wv, bv, Wo, bo):
    from concourse.bass_utils import run_bass_kernel_spmd

    nc = _program()
    in_maps = make_in_maps(inputs, context, Wq, bq, Wk, bk, Wv, bv, Wo, bo)
    res = run_bass_kernel_spmd(nc, in_maps, list(range(NCORES)))
    outs = [res.results[i]["out"] for i in range(NCORES)]
    bv = _f32(bv)
    Wo = _f32(Wo)
    bo = _f32(bo)
    corr = (bv.astype(np.float64) @ Wo.astype(np.float64)
            + bo.astype(np.float64)).astype(np.float32)
    full = np.stack([
        outs[0] + outs[1] + outs[2] + outs[3],
        outs[4] + outs[5] + outs[6] + outs[7],
    ]) + corr
    return full.astype(np.float32)


# revision 14
# speedup vs baseline: 2.3162x; 1.8899x over previous
"""Cross-attention Trainium2 kernel (8 NeuronCores).

Sharding: batch (2) x head-groups (4 groups of 4 heads) = 8 shards.
Each core computes q/k/v projections for its 4 heads (256 cols of
Wq/Wk/Wv), attention for those heads, and a partial out-projection
through its 256 rows of Wo.  The host sums the 4 partial outputs per
batch (the reduction of the head-parallel out_proj) and adds the
bv @ Wo + bo correction, which commutes exactly through the softmax
average.

Layout strategy on-core (v4):
  - x/ctx arrive HOST-TRANSPOSED (xT: [d, s]) so no PE transposes are
    needed; projections emit qT/kT ([head_dim, s], head pairs stacked
    even/odd on partitions 0-63/64-127) and v natural ([sk, head, hd]
    with a ones column for the softmax denominator).
  - scores are computed transposed (ST = k @ qT -> [sk, sq]); the
    even/odd head matmuls (K=64) are emitted adjacently so their
    auto-derived tile_positions (0,0)/(64,0) run concurrently on the
    two PE row-halves.
  - attention runs "orientation B": stationary = v(+ones) [sk,68],
    moving = the exp tile [sk,512].  The output IS attn^T [hd, sq]
    (what out_proj needs) with the softmax denominator in partition
    64; stationary loads are small and fully hidden under the 512-col
    moving streams.  Normalization = Pool partition_broadcast of the
    denominator row + DVE reciprocal + DVE multiply; odd heads reach
    partitions 64-127 through a tiny identity matmul (engines cannot
    write across partitions).
  - emission is paced by ACT (exp), the bottleneck engine (~134us
    busy): the prologue projects pair-0 columns first and spreads the
    first pair's scores between projection chunks so ACT starts at
    ~14us and never starves; each head-pair phase interleaves its
    attention matmuls with the NEXT pair's score matmuls (exp-tile
    liveness stays at one pair / 36 tiles); the last phase fuses both
    sq halves into one ACT-paced sweep (borrowing the projection PSUM
    pool) so only normalization + out_proj remain after the final exp.
"""

import numpy as np

import concourse.bass as bass
import concourse.mybir as mybir
import concourse.tile as tile
from concourse import bacc

B, SQ, SK, D, H, HS = 2, 2048, 2048, 1024, 16, 64
SCALE = HS ** -0.5
NCORES = 8
HG = 4            # heads per core
DG = HG * HS      # 256 projection cols per core

F32 = mybir.dt.float32
F16 = mybir.dt.float16
BF16 = mybir.dt.bfloat16


def build_program(loop_iters: int = 0):
    """Build the per-core SPMD Bass program."""
    nc = bacc.Bacc(None, target_bir_lowering=False, debug=False,
                   num_devices=NCORES)
    x_d = nc.dram_tensor("xT", [D, SQ], F16, kind="ExternalInput")
    c_d = nc.dram_tensor("cT", [D, SK], F16, kind="ExternalInput")
    wq_d = nc.dram_tensor("wq", [D, DG], F16, kind="ExternalInput")
    wk_d = nc.dram_tensor("wk", [D, DG], F16, kind="ExternalInput")
    wv_d = nc.dram_tensor("wv", [D, DG], F16, kind="ExternalInput")
    wo_d = nc.dram_tensor("wo", [DG, D], F16, kind="ExternalInput")
    bq_d = nc.dram_tensor("bq", [DG], F32, kind="ExternalInput")
    bk_d = nc.dram_tensor("bk", [DG], F32, kind="ExternalInput")
    i64_d = nc.dram_tensor("ident64", [64, 64], F16, kind="ExternalInput")
    out_d = nc.dram_tensor("out", [SQ, D], BF16, kind="ExternalOutput")

    with tile.TileContext(nc) as tc:
        with (
            tc.tile_pool(name="const", bufs=1) as cp,
            tc.tile_pool(name="persist", bufs=1) as psb,
            tc.tile_pool(name="xw", bufs=6) as xwp,
            tc.tile_pool(name="expp", bufs=40) as ep,
            tc.tile_pool(name="fin", bufs=6) as fpool,
            tc.tile_pool(name="outp", bufs=3) as opool,
            tc.tile_pool(name="pp", bufs=2, space="PSUM") as pp,
            tc.tile_pool(name="stp", bufs=2, space="PSUM") as stp,
            tc.tile_pool(name="atp", bufs=2, space="PSUM") as atp,
        ):
            import contextlib
            loop_ctx = tc.For_i(0, loop_iters, 1) if loop_iters else contextlib.nullcontext()
            loop_ctx.__enter__()

            ident64 = cp.tile([64, 64], F16, tag="ident64")
            wq_sb = cp.tile([128, 8, DG], F16, tag="wq")
            wk_sb = cp.tile([128, 8, DG], F16, tag="wk")
            wv_sb = cp.tile([128, 8, DG], F16, tag="wv")
            wo_sb = cp.tile([128, 2, D], F16, tag="wo")
            bq_sb = cp.tile([128, 2], F32, tag="bq")
            bk_sb = cp.tile([128, 2], F32, tag="bk")

            def load_weights_qx():
                nc.sync.dma_start(out=wq_sb, in_=wq_d[:].rearrange("(c p) n -> p c n", p=128))
                nc.sync.dma_start(out=bq_sb, in_=bq_d[:].rearrange("(c p) -> p c", p=128))

            def load_weights_k():
                nc.sync.dma_start(out=wk_sb, in_=wk_d[:].rearrange("(c p) n -> p c n", p=128))
                nc.sync.dma_start(out=bk_sb, in_=bk_d[:].rearrange("(c p) -> p c", p=128))
                nc.sync.dma_start(out=ident64, in_=i64_d[:])

            def load_weights_v():
                nc.sync.dma_start(out=wv_sb, in_=wv_d[:].rearrange("(c p) n -> p c n", p=128))

            def load_weights_o():
                nc.sync.dma_start(out=wo_sb, in_=wo_d[:].rearrange("(c p) n -> p c n", p=128))

            # persistent activations: qT/kT hold head pairs stacked on
            # partitions (even head p0-63, odd p64-127), pair index on the
            # middle axis, full sq/sk on the free axis
            qT = psb.tile([128, 2, SQ], F16, tag="qT", name="qT")
            kT = psb.tile([128, 2, SK], F16, tag="kT", name="kT")
            # v natural: [sk-chunk part, skc, head, 64+ones]
            vA = psb.tile([128, 16, HG, 68], F16, tag="vA", name="vA")
            # attn^T per sq-window: [pair-stacked head dim, pair, sq]
            aTw = [psb.tile([128, 2, 1024], F16, tag=f"aTw{s}", name=f"aTw{s}")
                   for s in range(2)]

            nc.vector.memset(vA[:], 1.0)

            cws = {}

            def proj_x(w, cs, dma=False, after_dma=None):
                if dma:
                    xw = xwp.tile([128, 8, 512], F16, tag="xw")
                    cws[("x", w)] = xw
                    nc.sync.dma_start(
                        out=xw,
                        in_=x_d[:, w * 512:(w + 1) * 512]
                            .rearrange("(c p) s -> p c s", p=128))
                    if after_dma is not None:
                        after_dma()
                xw = cws[("x", w)]
                for c in cs:
                    pq = pp.tile([128, 512], F32, tag="pp")
                    for dc in range(8):
                        nc.tensor.matmul(
                            pq,
                            (wq_sb[:, dc, c * 128:(c + 1) * 128]),
                            (xw[:, dc, :]),
                            start=(dc == 0), stop=(dc == 7),
                        )
                    nc.vector.tensor_scalar_add(
                        qT[:, c, w * 512:(w + 1) * 512], pq, bq_sb[:, c:c + 1])
                if cs[-1] == 1:
                    del cws[("x", w)]

            def proj_k(w, cs, dma=False, after_dma=None):
                if dma:
                    cw = xwp.tile([128, 8, 512], F16, tag="xw")
                    cws[("c", w)] = cw
                    nc.sync.dma_start(
                        out=cw,
                        in_=c_d[:, w * 512:(w + 1) * 512]
                            .rearrange("(c p) s -> p c s", p=128))
                    if after_dma is not None:
                        after_dma()
                cw = cws[("c", w)]
                for c in cs:
                    pk = pp.tile([128, 512], F32, tag="pp")
                    for dc in range(8):
                        nc.tensor.matmul(
                            pk,
                            (wk_sb[:, dc, c * 128:(c + 1) * 128]),
                            (cw[:, dc, :]),
                            start=(dc == 0), stop=(dc == 7),
                        )
                    nc.vector.tensor_scalar_add(
                        kT[:, c, w * 512:(w + 1) * 512], pk, bk_sb[:, c:c + 1])

            def proj_v(w, s4s):
                cw = cws[("c", w)]
                for s4 in s4s:
                    # runs inside phase-0 part1: projection psum pool is
                    # free there (atp holds the attn accumulators)
                    pv = pp.tile([128, 512], F32, tag="pp")
                    for dc in range(8):
                        nc.tensor.matmul(
                            pv[:, :DG],
                            (cw[:, dc, s4 * 128:(s4 + 1) * 128]),
                            (wv_sb[:, dc, :]),
                            start=(dc == 0), stop=(dc == 7),
                        )
                    nc.vector.tensor_copy(
                        vA[:, w * 4 + s4, :, 0:64],
                        pv[:, :DG].rearrange("p (h e) -> p h e", e=64),
                    )
                if s4s[-1] == 3:
                    del cws[("c", w)]

            # scores + exp for head pair t, sq window sqw, one sk chunk.
            # even/odd half-array matmuls are emitted adjacently so their
            # tile_positions (0,0)/(64,0) run concurrently on HW.
            def emit_se(t, sqw, skc):
                sts = [stp.tile([128, 1024], F32, tag="st", name=f"st{p}")
                       for p in range(2)]
                for half in range(2):
                    for par in range(2):
                        p0 = 64 * par
                        nc.tensor.matmul(
                            sts[par][:, half * 512:(half + 1) * 512],
                            (kT[p0:p0 + 64, t, skc * 128:(skc + 1) * 128]),
                            (qT[p0:p0 + 64, t,
                                sqw * 1024 + half * 512:
                                sqw * 1024 + (half + 1) * 512]),
                            start=True, stop=True,
                        )
                exs = []
                for par in range(2):
                    ex = ep.tile([128, 1024], F16, tag="ex")
                    nc.scalar.activation(
                        ex, sts[par], mybir.ActivationFunctionType.Exp,
                        scale=SCALE)
                    exs.append(ex)
                return exs

            # one attention accumulation matmul: stationary v(+ones),
            # moving a 512-wide half of the exp tile
            def mm_at(at, ex, h, skc, half):
                nc.tensor.matmul(
                    at[0:68, :],
                    vA[:, skc, h, :],
                    ex[:, half * 512:(half + 1) * 512],
                    start=(skc == 0), stop=(skc == 15),
                )

            # normalize one head's attn^T half and place it in aTw
            def norm(t, sqw, par, half, at, shift_pool=None):
                # DVE reciprocal of the denominator row (Pool cannot read
                # PSUM), then Pool broadcasts it across the 64 partitions
                rcrow = fpool.tile([1, 512], F32, tag="rcrow")
                nc.vector.reciprocal(rcrow, at[64:65, :])
                rc = fpool.tile([64, 512], F32, tag="rc")
                nc.gpsimd.partition_broadcast(rc, rcrow)
                dst = aTw[sqw][64 * par:64 * par + 64, t,
                               half * 512:(half + 1) * 512]
                if par == 0:
                    nc.vector.tensor_mul(dst, at[0:64, :], rc)
                else:
                    # engines cannot shift partitions; bounce through the
                    # PE with a small identity matmul
                    tmp = fpool.tile([64, 512], F16, tag="atmp")
                    nc.vector.tensor_mul(tmp, at[0:64, :], rc)
                    sp, sptag = shift_pool or (pp, "pp")
                    ps = sp.tile([128, 512], F32, tag=sptag, name="ps")
                    nc.tensor.matmul(ps[64:128, :], ident64, tmp,
                                     start=True, stop=True)
                    nc.vector.tensor_copy(dst, ps[64:128, :])

            # partial out-projection for one 128-row sq chunk; in the tail
            # (after the last exp) ACT is idle, so split the PSUM drains
            # between DVE and ScalarE there
            def emit_out_proj(sqc, use_act=False, po_pool=None):
                ot = opool.tile([128, D], BF16, tag="ot")
                sqw, c8 = sqc // 8, sqc % 8
                opl, optag = po_pool or (pp, "pp")
                for n2 in range(2):
                    po = opl.tile([128, 512], F32, tag=optag)
                    for kc in range(2):
                        nc.tensor.matmul(
                            po,
                            (aTw[sqw][:, kc, c8 * 128:(c8 + 1) * 128]),
                            (wo_sb[:, kc, n2 * 512:(n2 + 1) * 512]),
                            start=(kc == 0), stop=(kc == 1),
                        )
                    if use_act and n2 == 1:
                        nc.scalar.copy(ot[:, n2 * 512:(n2 + 1) * 512], po)
                    else:
                        nc.vector.tensor_copy(
                            ot[:, n2 * 512:(n2 + 1) * 512], po)
                nc.sync.dma_start(
                    out=out_d[sqc * 128:(sqc + 1) * 128, :], in_=ot)

            # ---- prologue: pair-0 projection columns first, with the
            # first pair's scores spread between projection chunks so ACT
            # starts early and stays fed
            P = [(0, 0), (1, 0), (0, 1), (1, 1)]
            e = {}
            se = {}
            proj_x(0, [0], dma=True, after_dma=load_weights_qx)
            proj_k(0, [0], dma=True, after_dma=load_weights_k)
            proj_x(1, [0], dma=True)
            se[0] = emit_se(0, 0, 0)
            se[1] = emit_se(0, 0, 1)
            proj_x(0, [1])
            se[2] = emit_se(0, 0, 2)
            proj_x(1, [1])
            se[3] = emit_se(0, 0, 3)
            proj_k(1, [0], dma=True, after_dma=load_weights_v)
            se[4] = emit_se(0, 0, 4)
            proj_k(0, [1])
            se[5] = emit_se(0, 0, 5)
            proj_v(0, [0, 1])
            se[6] = emit_se(0, 0, 6)
            proj_v(0, [2, 3])
            se[7] = emit_se(0, 0, 7)
            proj_x(2, [0, 1], dma=True)
            proj_k(2, [0], dma=True)
            se[8] = emit_se(0, 0, 8)
            proj_k(1, [1])
            se[9] = emit_se(0, 0, 9)
            proj_v(1, [0, 1])
            se[10] = emit_se(0, 0, 10)
            proj_v(1, [2, 3])
            se[11] = emit_se(0, 0, 11)
            proj_x(3, [0, 1], dma=True)
            proj_k(3, [0], dma=True, after_dma=load_weights_o)
            se[12] = emit_se(0, 0, 12)
            proj_k(2, [1])
            se[13] = emit_se(0, 0, 13)
            proj_v(2, [0, 1, 2, 3])
            se[14] = emit_se(0, 0, 14)
            proj_k(3, [1])
            se[15] = emit_se(0, 0, 15)
            proj_v(3, [0, 1, 2, 3])
            e[P[0]] = [se[i] for i in range(16)]

            # ---- main: 4 head-pair phases, ACT-paced
            for i in range(4):
                t, sqw = P[i]
                exE = [a for a, _ in e[P[i]]]
                exO = [b for _, b in e[P[i]]]
                atE = atp.tile([128, 512], F32, tag="at", name="atE")
                atO = atp.tile([128, 512], F32, tag="at", name="atO")
                if i < 3:
                    # part 1: first sq half, ACT-paced
                    for skc in range(16):
                        mm_at(atE, exE[skc], 2 * t, skc, 0)
                        mm_at(atO, exO[skc], 2 * t + 1, skc, 0)
                    norm(t, sqw, 0, 0, atE)
                    norm(t, sqw, 1, 0, atO)
                    se = [emit_se(*P[i + 1], 0), emit_se(*P[i + 1], 1)]
                    if i == 1:
                        for sqc in range(0, 4):
                            emit_out_proj(sqc)
                    # part 2: second sq half, weaving the next pair's scores
                    atE2 = atp.tile([128, 512], F32, tag="at", name="atE2")
                    atO2 = atp.tile([128, 512], F32, tag="at", name="atO2")
                    for skc in range(16):
                        mm_at(atE2, exE[skc], 2 * t, skc, 1)
                        mm_at(atO2, exO[skc], 2 * t + 1, skc, 1)
                        if skc < 14:
                            se.append(emit_se(*P[i + 1], skc + 2))
                    norm(t, sqw, 0, 1, atE2)
                    norm(t, sqw, 1, 1, atO2)
                    e[P[i + 1]] = se
                    if i == 1:
                        for sqc in range(4, 8):
                            emit_out_proj(sqc)
                else:
                    # final phase: both sq halves in one ACT-paced sweep
                    # (the projection PSUM pool is free here), so only
                    # normalization + out_proj trail the last exp
                    atE2 = pp.tile([128, 512], F32, tag="pp", name="atE2")
                    atO2 = pp.tile([128, 512], F32, tag="pp", name="atO2")
                    for skc in range(16):
                        mm_at(atE, exE[skc], 2 * t, skc, 0)
                        mm_at(atO, exO[skc], 2 * t + 1, skc, 0)
                        mm_at(atE2, exE[skc], 2 * t, skc, 1)
                        mm_at(atO2, exO[skc], 2 * t + 1, skc, 1)
                    # half-a norms first (shift psum + out_proj psums borrow
                    # the freed atp slots) so out_proj(8..11) overlaps the
                    # half-b norms
                    norm(t, sqw, 0, 0, atE)
                    norm(t, sqw, 1, 0, atO, shift_pool=(atp, "at"))
                    for sqc in range(8, 12):
                        emit_out_proj(sqc, use_act=True, po_pool=(atp, "at"))
                    norm(t, sqw, 0, 1, atE2)
                    norm(t, sqw, 1, 1, atO2)
                    for sqc in range(12, 16):
                        emit_out_proj(sqc, use_act=True)
            loop_ctx.__exit__(None, None, None)

    nc.compile()
    return nc


_NC = None


def _program():
    global _NC
    if _NC is None:
        _NC = build_program()
    return _NC


def _f32(a):
    return np.ascontiguousarray(np.asarray(a, dtype=np.float32))


def make_in_maps(inputs, context, Wq, bq, Wk, bk, Wv, bv, Wo, bo):
    inputs = np.asarray(inputs)
    context = np.asarray(context)
    Wq, bq, Wk, bk = (np.asarray(a) for a in (Wq, bq, Wk, bk))
    Wv, Wo = np.asarray(Wv), np.asarray(Wo)
    in_maps = []
    for core in range(NCORES):
        b, g = core // HG, core % HG
        sl = slice(DG * g, DG * (g + 1))
        in_maps.append({
            "xT": np.ascontiguousarray(inputs[b].T.astype(np.float16)),
            "cT": np.ascontiguousarray(context[b].T.astype(np.float16)),
            "wq": np.ascontiguousarray(Wq[:, sl].astype(np.float16)),
            "wk": np.ascontiguousarray(Wk[:, sl].astype(np.float16)),
            "wv": np.ascontiguousarray(Wv[:, sl].astype(np.float16)),
            "wo": np.ascontiguousarray(Wo[sl, :].astype(np.float16)),
            "bq": _f32(bq[sl]),
            "bk": _f32(bk[sl]),
            "ident64": np.eye(64, dtype=np.float16),
        })
    return in_maps


def kernel(inputs, context, Wq, bq, Wk, bk, Wv, bv, Wo, bo):
    from concourse.bass_utils import run_bass_kernel_spmd

    nc = _program()
    in_maps = make_in_maps(inputs, context, Wq, bq, Wk, bk, Wv, bv, Wo, bo)
    res = run_bass_kernel_spmd(nc, in_maps, list(range(NCORES)))
    outs = [np.asarray(res.results[i]["out"]).astype(np.float32)
            for i in range(NCORES)]
    bv = _f32(bv)
    Wo = _f32(Wo)
    bo = _f32(bo)
    corr = (bv.astype(np.float64) @ Wo.astype(np.float64)
            + bo.astype(np.float64)).astype(np.float32)
    full = np.stack([
        outs[0] + outs[1] + outs[2] + outs[3],
        outs[4] + outs[5] + outs[6] + outs[7],
    ]) + corr
    return full.astype(np.float32)
